# revision 17
# baseline (speedup 1.0000x reference)
"""Trainium2 Bass kernel for nn_LogicAutoEncoder.

Math: board_state (B,9,3) one-hot -> logits (B,9,3).
  sim[b,r,p,i] depends on the board only through cell state c = state(b,i),
  so sim = T[r,p,i,c] (a 432-entry table, computed on host) and
    val[b,(rp,i)] = board_onehot[b] @ W        (table-lookup as matmul)
    sat = max_i val;  act = prod_p sat;  out = act @ heads + bias.

Device pipeline (pure data parallel over 8 cores, 65536 rows each), per
4096-row supertile (partition p holds 32 consecutive rows of 27 floats):
  1. one contiguous DMA in (128, 864)
  2. PE transposes 4-slice blocks (128,108) -> PSUM (108,128); ScalarE
     copies to SBUF (the matmul stationary operand must be SBUF)
  3. PE block-diag matmul pairs of slices: (108,128)^T @ W2 (108,576 block
     diag) -> out1 (128, 2x288) in PSUM  [float32r: 1 cyc/row, ~1.2e-4 rel]
  4. DVE reduce_max over i (9) on 2-pair PSUM units -> sat; TT mul -> act
  5. PE transposes act in 10/10/12-slice groups, ScalarE copy, PE
     block-diag heads matmul (bias folded in via an appended ones column)
  6. ScalarE copy -> SBUF, one contiguous DMA out (SWDGE queue).
"""

import os
import sys
import functools

import numpy as np

sys.path.insert(0, "/opt/trn_rl_repo")

B = 524288
N_CORES = 8
BC = B // N_CORES            # 65536 rows per core
ST_ROWS = 4096               # rows per supertile
N_ST = BC // ST_ROWS         # 16 supertiles
SLICES = 32                  # row-slices per supertile
NF = 144                     # features per slice: 16 premises x 9 cells
OUT_D = 27
HGRP = [(0, 10), (10, 10), (20, 12)]  # heads-stage slice groups (even N)

MM_DT_NAME = os.environ.get("KERNEL_MM_DT", "float32r")


def _build_program():
    import concourse.bacc as bacc
    import concourse.mybir as mybir
    import concourse.tile as tile

    f32 = mybir.dt.float32
    mm_dt = getattr(mybir.dt, MM_DT_NAME)

    nc = bacc.Bacc(
        "TRN2", target_bir_lowering=False, debug=False, num_devices=N_CORES
    )
    bs_d = nc.dram_tensor("bs", [BC, 27], mm_dt, kind="ExternalInput")
    w2_d = nc.dram_tensor("w2", [108, 576], mm_dt, kind="ExternalInput")
    hba_d = nc.dram_tensor("hba", [90, 270], mm_dt, kind="ExternalInput")
    hbb_d = nc.dram_tensor("hbb", [108, 324], mm_dt, kind="ExternalInput")
    idm_d = nc.dram_tensor("idm", [128, 128], mm_dt, kind="ExternalInput")
    idmf_d = nc.dram_tensor("idmf", [128, 128], f32, kind="ExternalInput")
    out_d = nc.dram_tensor("out", [BC, 27], f32, kind="ExternalOutput")

    bs_v = bs_d.rearrange("(s p k) f -> s p (k f)", s=N_ST, p=128, k=SLICES)
    out_v = out_d.rearrange("(s p k) f -> s p (k f)", s=N_ST, p=128, k=SLICES)

    with tile.TileContext(nc) as tc:
        with (
            tc.tile_pool(name="singles", bufs=1) as singles,
            tc.tile_pool(name="bs_in", bufs=3) as bs_pool,
            tc.tile_pool(name="bsT_sb", bufs=2) as bsT_pool,
            tc.tile_pool(name="sat", bufs=2) as sat_pool,
            tc.tile_pool(name="act", bufs=2) as act_pool,
            tc.tile_pool(name="actT_sb", bufs=2) as actT_pool,
            tc.tile_pool(name="out_sb", bufs=3) as out_pool,
            tc.tile_pool(name="p_bsT", bufs=1, space="PSUM") as p_bsT,
            tc.tile_pool(name="p_o1", bufs=2, space="PSUM") as p_o1,
            tc.tile_pool(name="p_actT", bufs=1, space="PSUM") as p_actT,
            tc.tile_pool(name="p_o2", bufs=2, space="PSUM") as p_o2,
        ):
            w2_sb = singles.tile([108, 576], mm_dt)
            nc.gpsimd.dma_start(out=w2_sb[:], in_=w2_d[:])
            hba_sb = singles.tile([90, 270], mm_dt)
            nc.gpsimd.dma_start(out=hba_sb[:], in_=hba_d[:])
            hbb_sb = singles.tile([108, 324], mm_dt)
            nc.gpsimd.dma_start(out=hbb_sb[:], in_=hbb_d[:])
            idm_sb = singles.tile([128, 128], mm_dt)
            nc.gpsimd.dma_start(out=idm_sb[:], in_=idm_d[:])
            idmf_sb = singles.tile([128, 128], f32)
            nc.gpsimd.dma_start(out=idmf_sb[:], in_=idmf_d[:])

            for st in range(N_ST):
                bs_in = bs_pool.tile([128, SLICES * 27], mm_dt)
                nc.sync.dma_start(out=bs_in[:], in_=bs_v[st])

                # transpose groups of 4 slices: (128,108) -> (108,128)
                bsT_sbs = []
                for t in range(2):  # two (108, 512) psum tiles
                    pt = p_bsT.tile([108, 512], mm_dt)
                    for gg in range(4):
                        g = t * 4 + gg
                        nc.tensor.transpose(
                            pt[:, gg * 128 : (gg + 1) * 128],
                            bs_in[:, g * 108 : (g + 1) * 108],
                            idm_sb[:],
                        )
                    sb = bsT_pool.tile([108, 512], mm_dt)
                    nc.scalar.copy(sb[:], pt[:])
                    bsT_sbs.append(sb)

                sat = sat_pool.tile([128, SLICES * 16], f32)
                for u in range(8):  # 2 slice-pairs per unit
                    o1 = p_o1.tile([128, 1024], f32)
                    for c in range(2):
                        j = 2 * u + c  # slice pair (2j, 2j+1)
                        g = j // 2
                        t, gg = g // 4, g % 4
                        lhsT = bsT_sbs[t][:, gg * 128 : (gg + 1) * 128]
                        rhs = w2_sb[:, (j % 2) * 288 : (j % 2 + 1) * 288]
                        nc.tensor.matmul(
                            o1[:, c * 512 : c * 512 + 288],
                            lhsT,
                            rhs,
                            start=True,
                            stop=True,
                        )
                    o1v = o1[:].rearrange("a (c rest) -> a c rest", c=2)
                    nc.vector.reduce_max(
                        sat[:, u * 64 : (u + 1) * 64],
                        o1v[:, :, 0:288].rearrange("a c (g i) -> a c g i", i=9),
                        axis=mybir.AxisListType.X,
                    )

                # act[:, sl, r] = sat[:,sl,0,r]*sat[:,sl,1,r]; act[:, sl, 8]=1
                act = act_pool.tile([128, SLICES, 9], f32)
                sat3 = sat[:].rearrange("a (sl p8 r) -> a (sl p8) r", p8=2, r=8)
                nc.gpsimd.memset(act[:, :, 8:9], 1.0)
                nc.gpsimd.tensor_mul(
                    act[:, :, 0:8],
                    sat3[:, 0::2, :],
                    sat3[:, 1::2, :],
                )

                out_sb = out_pool.tile([128, SLICES * 27], f32)
                act2 = act[:].rearrange("a sl r -> a (sl r)")
                pa = p_actT.tile([108, 384], f32)
                for gi, (s0, ns) in enumerate(HGRP):
                    nc.tensor.transpose(
                        pa[0 : ns * 9, gi * 128 : (gi + 1) * 128],
                        act2[:, s0 * 9 : (s0 + ns) * 9],
                        idmf_sb[:],
                    )
                aT = actT_pool.tile([108, 384], mm_dt)
                nc.scalar.copy(aT[:], pa[:])
                for gi, (s0, ns) in enumerate(HGRP):
                    hb = hba_sb if ns == 10 else hbb_sb
                    po2 = p_o2.tile([128, 512], f32)
                    nc.tensor.matmul(
                        po2[:, 0 : ns * 27],
                        aT[0 : ns * 9, gi * 128 : (gi + 1) * 128],
                        hb[:],
                        start=True,
                        stop=True,
                    )
                    nc.scalar.copy(
                        out_sb[:, s0 * 27 : (s0 + ns) * 27], po2[:, 0 : ns * 27]
                    )

                nc.gpsimd.dma_start(out=out_v[st], in_=out_sb[:])

    nc.compile()
    return nc


@functools.cache
def _get_program():
    return _build_program()


def _host_tables(premises, heads, bias):
    """Build the block-diag lookup tables on host (tiny)."""
    pos = (np.arange(9, dtype=np.float64) - 4.0) / 4.0
    pl = np.array([0.0, 1.0, -1.0], dtype=np.float64)
    prem = premises.astype(np.float64)
    d_pl = (pl[None, None, :] - prem[:, :, 0][:, :, None]) ** 2  # (8,2,3)
    d_pos = (pos[None, None, :] - prem[:, :, 1][:, :, None]) ** 2  # (8,2,9)
    T = np.exp(-(d_pl[:, :, None, :] + d_pos[:, :, :, None]))  # (8,2,9,3)

    wtab = np.zeros((27, NF), dtype=np.float32)  # [(i,c), (p8,r,i)]
    for r in range(8):
        for p8 in range(2):
            for i in range(9):
                for c in range(3):
                    wtab[i * 3 + c, p8 * 72 + r * 9 + i] = T[r, p8, i, c]
    w2 = np.zeros((108, 576), dtype=np.float32)
    for u in range(4):
        w2[u * 27 : (u + 1) * 27, u * 144 : (u + 1) * 144] = wtab

    heads9 = np.zeros((9, 27), dtype=np.float32)
    heads9[0:8] = heads.astype(np.float32)
    heads9[8] = bias.astype(np.float32)
    hba = np.zeros((90, 270), dtype=np.float32)
    for v in range(10):
        hba[v * 9 : (v + 1) * 9, v * 27 : (v + 1) * 27] = heads9
    hbb = np.zeros((108, 324), dtype=np.float32)
    for v in range(12):
        hbb[v * 9 : (v + 1) * 9, v * 27 : (v + 1) * 27] = heads9
    return w2, hba, hbb


def kernel(board_state, premises, heads, bias):
    from concourse.bass_utils import run_bass_kernel_spmd

    nc = _get_program()
    w2, hba, hbb = _host_tables(
        np.asarray(premises), np.asarray(heads), np.asarray(bias)
    )
    idm = np.eye(128, dtype=np.float32)

    bs_flat = np.ascontiguousarray(board_state, dtype=np.float32).reshape(B, 27)
    in_maps = []
    for k in range(N_CORES):
        in_maps.append(
            {
                "bs": bs_flat[k * BC : (k + 1) * BC],
                "w2": w2,
                "hba": hba,
                "hbb": hbb,
                "idm": idm,
                "idmf": idm,
            }
        )
    res = run_bass_kernel_spmd(
        nc,
        in_maps,
        core_ids=list(range(N_CORES)),
        trace=bool(int(os.environ.get("KERNEL_TRACE", "0"))),
    )
    out = np.concatenate([r["out"] for r in res.results], axis=0)
    kernel.last_results = res
    return out.reshape(B, 9, 3).astype(np.float32)
